# revision 1
# baseline (speedup 1.0000x reference)
"""Causal GRN-EMA normalization kernel for 8x TRN2 NeuronCores (Bass/Tile).

Math (per batch b, channel c, time t):
    ema_t   = ALPHA*ema_{t-1} + (1-ALPHA)*x_t^2,  ema_{-1} = EMA_INIT
    ema_hat = ema_t / (1 - ALPHA^{t+1} + EPS)
    g       = sqrt(ema_hat + EPS)
    n       = g / (mean_c(g) + EPS)
    y       = gamma*(x*n) + beta + x

Device strategy (data-parallel over B, 2 batches/core):
  - x is shipped in bf16; the device computes n' = g / sum_c(g) in bf16;
    the host applies y = x*(1 + (C*gamma)*n') + beta in f32 (exact affine,
    same spirit as the baseline's host-side beta/rotation).
  - The T-recurrence is a blocked scan: per 128-step block,
        within[i,c] = sum_{j<=i} (1-A)*A^(i-j) * x[j,c]^2   (lower-tri matmul)
        ema[i,c]    = within[i,c] + A^(i+1) * E_k[c]        (K=33 matmul)
    and the block carries E_k for a sub-batch of SB=16 blocks are produced
    in one shot by accumulating per-block "decay-weighted carry" matmuls
    into an Emat psum tile ([33,512]: rows 0..15 = E_k, row 32 = S_next),
    chained across sub-batches by a K=33 matmul on the previous esb tile.
    This removes both the serial per-block carry chain and the per-block
    PSUM->SBUF row copy of the baseline.
  - Engine balance per block: PE within+Eadd+Dmat (3x213ns), ACT sqrt
    (612ns), DVE square/4 + n + recip, Pool channel-sum + esb copies.
"""

import os
from contextlib import ExitStack

import numpy as np

ALPHA = 0.99
EPS = 1e-6
EMA_INIT = 1e-4

B, T, C = 16, 8192, 512
NCORES = 8
BPC = B // NCORES          # batches per core
L = 128                    # scan block (partition dim)
NBLK = T // L              # 64 blocks per batch

DEFAULT_CFG = dict(
    win_sbs=(16, 16, 16, 16),  # per-batch sub-batch (carry window) sizes
    chunk=4,           # blocks per DMA chunk
    # per-chunk pattern: which block's channel-sum runs on DVE (else ACT
    # accum_out on the sqrt) — Pool supports neither PSUM nor TSP-accum
    s_dve=(0, 1, 2, 3),
    # which chunk squares run where: index%len -> engine
    sq_pat=("dve", "pool", "dve", "dve"),
    esb_copy="dve",    # "act" | "dve"  (Pool cannot access PSUM)
    y_dma="sp",        # engine queue for n-out DMAs: "sp" | "act"
    x_dma="act",       # engine queue for x-in DMAs: "sp" | "act"
    xin_bufs=12,
    bsq_bufs=3,
    g_bufs=3,
    nt_bufs=3,
    st_bufs=4,
    esb_bufs=3,
    pblk_bufs=3,
    emat_bufs=1,
    prefetch_head=2,
    prefetch_post=0,   # extra chunks prefetched right after the constants
    depth=2,
    warmup=True,
    warmup_n=4,
    x_sp_head=0,       # first N x-chunk DMAs ride the SP queue (empty early)
    interleave_b=False,
    p1_pre=0,          # x+square chunks of wi+2 issued before the esb copy
    p3_first=False,
    y_split=1,         # split each n-out DMA into this many pieces
)

_cache = {}


def _host_constants(sbmax, sb_sizes):
    i = np.arange(L, dtype=np.float64)
    jj, ii = np.meshgrid(i, i, indexing="ij")
    # within-scan weights: lmatT[j, i] = (1-A)*A^(i-j) for j <= i
    lmatT = np.where(jj <= ii, (1.0 - ALPHA) * ALPHA ** (ii - jj), 0.0)
    # carry weights: w_m[c] = sum_j cw[j] * bsq_m[j, c]
    cw = (1.0 - ALPHA) * ALPHA ** (L - 1 - i)
    a128 = ALPHA**L

    def make_dmW(sb):
        # dmW[:, 33*m + k]: contribution of bsq_m to Emat row k
        #   rows 0..sb-1 hold E_k (k>m terms), row 32 holds S_next
        dmW = np.zeros((L, sb * 33))
        for m in range(sb):
            for k in range(m + 1, sb):
                dmW[:, 33 * m + k] = a128 ** (k - 1 - m) * cw
            dmW[:, 33 * m + 32] = a128 ** (sb - 1 - m) * cw
        return dmW

    def make_srowT(sb):
        # S-chain: Emat[k] += A^(128k)*esb_prev[32]; lhsT [33,33] row 32 only
        srowT = np.zeros((33, 33))
        for k in range(sb):
            srowT[32, k] = a128**k
        srowT[32, 32] = a128**sb
        return srowT

    dmWs = {sb: make_dmW(sb) for sb in sorted(set(sb_sizes))}
    srowTs = {sb: make_srowT(sb) for sb in sorted(set(sb_sizes))}
    # E-add: psum[i,c] += powv[i] * esb[k, c];  EW[:, 128*k+i] one-hot in k
    powv = ALPHA ** (i + 1)
    EW = np.zeros((33, sbmax * L))
    for k in range(sbmax):
        EW[k, 128 * k : 128 * (k + 1)] = powv
    # rden[p, kblk] = 1 / (1 - A^(128*kblk + p + 1) + EPS)
    kb = np.arange(NBLK, dtype=np.float64)
    tg = L * kb[None, :] + i[:, None] + 1.0
    rden = 1.0 / (1.0 - ALPHA**tg + EPS)
    f32 = np.float32
    cat = lambda ds: np.ascontiguousarray(
        np.concatenate([ds[sb] for sb in sorted(ds)], axis=1).astype(f32)
    )
    return (
        np.ascontiguousarray(lmatT.astype(f32)),
        cat(dmWs),
        cat(srowTs),
        np.ascontiguousarray(EW.astype(f32)),
        np.ascontiguousarray(rden.astype(f32)),
        {sb: j for j, sb in enumerate(sorted(set(sb_sizes)))},
    )


def _build_nc(cfg=None):
    import concourse.bacc as bacc
    import concourse.bass as bass
    import concourse.mybir as mybir
    import concourse.tile as tile

    cfg = {**DEFAULT_CFG, **(cfg or {})}
    CH = cfg["chunk"]
    win_sbs = cfg["win_sbs"]
    assert sum(win_sbs) == NBLK
    SBMAX = max(win_sbs)
    sizes = sorted(set(win_sbs))
    sb_off = {}   # column offsets into the concatenated dmW / srowT consts
    off = 0
    for sb in sizes:
        sb_off[sb] = off
        off += 33 * sb
    srow_off = {sb: 33 * j for j, sb in enumerate(sizes)}
    DMW_COLS = off

    # windows: (batch, first block, sb)
    wins = []
    if cfg["interleave_b"]:
        blk0s = [0] * BPC
        for sb in win_sbs:
            for b in range(BPC):
                wins.append((b, blk0s[b], sb))
                blk0s[b] += sb
    else:
        for b in range(BPC):
            blk0 = 0
            for sb in win_sbs:
                wins.append((b, blk0, sb))
                blk0 += sb
    NW = len(wins)
    # S-chain predecessor window (same batch), or None at a batch start
    prev_win = {}
    last_of = {}
    for wi, (b, blk0, sb) in enumerate(wins):
        prev_win[wi] = last_of.get(b)
        last_of[b] = wi
    nch = [sb // CH for (_, _, sb) in wins]

    f32 = mybir.dt.float32
    bf16 = mybir.dt.bfloat16

    nc = bacc.Bacc()
    x_h = nc.dram_tensor("x", [BPC, T, C], bf16, kind="ExternalInput")
    lmatT_h = nc.dram_tensor("lmatT", [L, L], bf16, kind="ExternalInput")
    dmW_h = nc.dram_tensor("dmW", [L, DMW_COLS], bf16, kind="ExternalInput")
    srowT_h = nc.dram_tensor(
        "srowT", [33, 33 * len(sizes)], bf16, kind="ExternalInput"
    )
    EW_h = nc.dram_tensor("EW", [33, SBMAX * L], bf16, kind="ExternalInput")
    rden_h = nc.dram_tensor("rden", [L, NBLK], f32, kind="ExternalInput")
    y_h = nc.dram_tensor("y", [BPC, T, C], bf16, kind="ExternalOutput")

    with tile.TileContext(nc) as tc, ExitStack() as ctx:
        singles = ctx.enter_context(tc.tile_pool(name="singles", bufs=1))
        xin = ctx.enter_context(tc.tile_pool(name="xin", bufs=cfg["xin_bufs"]))
        bsqp = ctx.enter_context(tc.tile_pool(name="bsqp", bufs=cfg["bsq_bufs"]))
        gp = ctx.enter_context(tc.tile_pool(name="gp", bufs=cfg["g_bufs"]))
        ntp = ctx.enter_context(tc.tile_pool(name="ntp", bufs=cfg["nt_bufs"]))
        stp = ctx.enter_context(tc.tile_pool(name="stp", bufs=cfg["st_bufs"]))
        esbp = ctx.enter_context(tc.tile_pool(name="esbp", bufs=cfg["esb_bufs"]))
        psum = ctx.enter_context(
            tc.tile_pool(name="psum", bufs=cfg["pblk_bufs"], space="PSUM")
        )
        ematp = ctx.enter_context(
            tc.tile_pool(name="ematp", bufs=cfg["emat_bufs"], space="PSUM")
        )
        if cfg["warmup"]:
            wpsum = ctx.enter_context(
                tc.tile_pool(name="wpsum", bufs=1, space="PSUM")
            )

        xeng = {"sp": nc.sync, "act": nc.scalar}[cfg["x_dma"]]
        xts = {}
        bsq = {}
        sq_state = [0]
        xfetches = [0]

        def fetch_chunk(wi, q):
            if (wi, q) in xts:
                return
            xt = xin.tile([L, CH, C], bf16, name=f"pf{wi}_{q}", tag="xt")
            eng = nc.sync if xfetches[0] < cfg["x_sp_head"] else xeng
            xfetches[0] += 1
            eng.dma_start(out=xt, in_=x_view(wi, q))
            xts[(wi, q)] = xt

        def alloc_bsq(wi):
            if wi not in bsq:
                bsq[wi] = bsqp.tile(
                    [L, SBMAX, C], bf16, name=f"bsq{wi}", tag="bsq"
                )

        def square_chunk(wi, q):
            """x^2 for chunk q on the engine given by sq_pat."""
            alloc_bsq(wi)
            xt = xts.pop((wi, q))
            eng = cfg["sq_pat"][sq_state[0] % len(cfg["sq_pat"])]
            sq_state[0] += 1
            bslice = bsq[wi][:, q * CH : (q + 1) * CH, :]
            if eng == "pool":
                nc.gpsimd.tensor_mul(bslice, xt, xt)
            elif eng == "act":
                nc.scalar.activation(
                    out=bslice, in_=xt,
                    func=mybir.ActivationFunctionType.Square,
                )
            else:
                nc.vector.tensor_mul(bslice, xt, xt)

        def bsq_ap(wi, blk):
            return bsq[wi][:, blk, :]

        def x_view(wi, q):
            b, blk0, sb = wins[wi]
            t0 = (blk0 + q * CH) * L
            return x_h[b, t0 : t0 + CH * L, :].rearrange("(n p) c -> p n c", p=L)

        def y_view(wi, q):
            b, blk0, sb = wins[wi]
            t0 = (blk0 + q * CH) * L
            return y_h[b, t0 : t0 + CH * L, :].rearrange("(n p) c -> p n c", p=L)

        # --- head prefetch: x^2 DMAs before the constants ---
        order = [(wi, q) for wi in range(NW) for q in range(nch[wi])]
        for wi, q in order[: cfg["prefetch_head"]]:
            fetch_chunk(wi, q)

        # --- constants ---
        lmatT_s = singles.tile([L, L], bf16)
        nc.sync.dma_start(out=lmatT_s, in_=lmatT_h[:, :])
        dmW_s = singles.tile([L, DMW_COLS], bf16)
        nc.sync.dma_start(out=dmW_s, in_=dmW_h[:, :])
        srowT_s = singles.tile([33, 33 * len(sizes)], bf16)
        nc.sync.dma_start(out=srowT_s, in_=srowT_h[:, :])
        EW_s = singles.tile([33, SBMAX * L], bf16)
        nc.sync.dma_start(out=EW_s, in_=EW_h[:, :])
        rden_s = singles.tile([L, NBLK], f32)
        nc.sync.dma_start(out=rden_s, in_=rden_h[:, :])
        eps_s = singles.tile([L, 1], f32)
        nc.vector.memset(eps_s, EPS)
        ph = cfg["prefetch_head"]
        for wi, q in order[ph : ph + cfg["prefetch_post"]]:
            fetch_chunk(wi, q)
        # esb_init: only row 32 (the S-chain seed) is consumed
        esb_init = singles.tile([33, C], bf16)
        nc.gpsimd.memset(esb_init, EMA_INIT)

        # --- engine warmups: absorb const-DMA waits into engine clocks and
        # pre-ramp the PE p-state before the first real matmuls ---
        if cfg["warmup"]:
            warm = [
                (lmatT_s[:, 0:1], lmatT_s[:, :]),
                (dmW_s[:, 0:1], dmW_s[:, 0:512]),
                (srowT_s[:, 0:1], srowT_s[:, :]),
                (EW_s[:, 0:1], EW_s[:, 0:512]),
            ]
            for wi_ in range(cfg["warmup_n"]):
                wl, wr = warm[wi_ % len(warm)]
                wup = wpsum.tile([L, 512], f32, tag="warmup", name=f"wup{wi_}")
                nc.tensor.matmul(
                    wup[: wl.shape[-1], : wr.shape[-1]], wl, wr,
                    start=True, stop=True,
                )
        scr_act = singles.tile([L, 1], f32)
        nc.scalar.copy(out=scr_act, in_=rden_s[:, 0:1])
        scr_dve = singles.tile([L, 1], f32)
        nc.vector.tensor_copy(out=scr_dve, in_=eps_s)
        scr_pool = singles.tile([L, 1], f32)
        nc.gpsimd.tensor_copy(out=scr_pool, in_=rden_s[:, 0:1])

        emat = {}
        esb = {}

        def start_P1(wi):
            """S-chain matmul opening window wi's Emat accumulation."""
            b, blk0, sb = wins[wi]
            alloc_bsq(wi)
            emat[wi] = ematp.tile([33, C], f32, name=f"emat{wi}", tag="emat")
            prev = esb_init if prev_win[wi] is None else esb[prev_win[wi]]
            so = srow_off[sb]
            nc.tensor.matmul(
                emat[wi], srowT_s[:, so : so + 33], prev[:, :],
                start=True, stop=False,
            )

        def P1_xsq(wi, q):
            """Stream chunk q of window wi's x, square into bsq."""
            fetch_chunk(wi, q)
            square_chunk(wi, q)

        def P1_dmw(wi, q):
            """Accumulate chunk q's carry (dmW) matmuls into Emat(wi)."""
            b, blk0, sb = wins[wi]
            do = sb_off[sb]
            for m in range(q * CH, (q + 1) * CH):
                nc.tensor.matmul(
                    emat[wi],
                    dmW_s[:, do + 33 * m : do + 33 * (m + 1)],
                    bsq_ap(wi, m),
                    start=False,
                    stop=(m == sb - 1),
                )

        def P1_chunk(wi, q):
            P1_xsq(wi, q)
            P1_dmw(wi, q)

        def finish_P1(wi):
            """Emat -> esb (bf16) copy closing window wi's carry batch."""
            esb[wi] = esbp.tile([33, C], bf16, name=f"esb{wi}", tag="esb")
            if cfg["esb_copy"] == "act":
                nc.scalar.copy(out=esb[wi], in_=emat[wi])
            elif cfg["esb_copy"] == "pooldma":
                nc.gpsimd.dma_start(out=esb[wi], in_=emat[wi][:, :])
            else:
                nc.vector.tensor_copy(out=esb[wi], in_=emat[wi])

        def P3_chunk(wi, q):
            """Normalize chunk q of window wi and DMA n out.

            Blocks with t >= 1024 have rden = 1/(1-A^t+EPS) within 3.4e-5
            of 1.0, so pairs of blocks share one sqrt over a [L,2,C] psum
            view with scale=1; early blocks get exact per-block sqrts.
            """
            b, blk0, sb = wins[wi]
            gt = gp.tile([L, CH, C], bf16, tag="gt")
            nt = ntp.tile([L, CH, C], bf16, tag="nt")
            st = stp.tile([L, CH], f32, tag="st")
            rm = stp.tile([L, CH], f32, tag="rm")
            exact = (blk0 + q * CH) * L < 1024
            for half in range(CH // 2):
                pb = psum.tile(
                    [L, 2, C], f32, tag="pblk", name=f"pb{wi}_{q}_{half}"
                )
                for j2 in range(2):
                    j = 2 * half + j2
                    blk = q * CH + j
                    nc.tensor.matmul(
                        pb[:, j2, :], lmatT_s[:, :], bsq_ap(wi, blk),
                        start=True, stop=False,
                    )
                    nc.tensor.matmul(
                        pb[:, j2, :],
                        EW_s[:, L * blk : L * (blk + 1)],
                        esb[wi][:, :],
                        start=False,
                        stop=True,
                    )
                if exact:
                    for j2 in range(2):
                        j = 2 * half + j2
                        kg = blk0 + q * CH + j
                        nc.scalar.activation(
                            out=gt[:, j, :],
                            in_=pb[:, j2, :],
                            func=mybir.ActivationFunctionType.Sqrt,
                            bias=eps_s,
                            scale=rden_s[:, kg : kg + 1],
                        )
                else:
                    nc.scalar.activation(
                        out=gt[:, 2 * half : 2 * half + 2, :],
                        in_=pb,
                        func=mybir.ActivationFunctionType.Sqrt,
                        bias=eps_s,
                    )
                for j2 in range(2):
                    j = 2 * half + j2
                    nc.vector.tensor_scalar(
                        out=gt[:, j, :], in0=gt[:, j, :], scalar1=1.0,
                        scalar2=0.0, op0=mybir.AluOpType.mult,
                        op1=mybir.AluOpType.add,
                        accum_out=st[:, j : j + 1],
                    )
            nc.vector.reciprocal(out=rm, in_=st)
            for j in range(CH):
                nc.vector.tensor_scalar(
                    out=nt[:, j, :], in0=gt[:, j, :], scalar1=rm[:, j : j + 1],
                    scalar2=None, op0=mybir.AluOpType.mult,
                )
            yeng = {"sp": nc.sync, "act": nc.scalar}[cfg["y_dma"]]
            ys = cfg["y_split"]
            step = CH // ys
            yv = y_view(wi, q)
            for p0 in range(0, CH, step):
                yeng.dma_start(
                    out=yv[:, p0 : p0 + step, :], in_=nt[:, p0 : p0 + step, :]
                )

        if cfg["depth"] == 2:
            # --- depth-2 pipeline: P1 runs two windows ahead of P3, so
            # esb(wi+1) is copied a full window before P3(wi+1) needs it ---
            start_P1(0)
            for q in range(nch[0]):
                P1_chunk(0, q)
            finish_P1(0)
            start_P1(1)
            for q in range(nch[1]):
                P1_chunk(1, q)
            PRE = cfg["p1_pre"]
            for wi in range(NW):
                # issue x+square for the first PRE chunks of wi+2 before the
                # esb copy so the copy never head-of-line blocks them on DVE
                npre = min(PRE, nch[wi + 2]) if wi + 2 < NW else 0
                for q in range(npre):
                    P1_xsq(wi + 2, q)
                if wi + 1 < NW:
                    finish_P1(wi + 1)
                if wi + 2 < NW:
                    start_P1(wi + 2)
                for q in range(npre):
                    P1_dmw(wi + 2, q)
                qs1 = list(range(npre, nch[wi + 2])) if wi + 2 < NW else []
                qs3 = list(range(nch[wi]))
                while qs1 or qs3:
                    if cfg["p3_first"]:
                        if qs3:
                            P3_chunk(wi, qs3.pop(0))
                        if qs1:
                            P1_chunk(wi + 2, qs1.pop(0))
                    else:
                        if qs1:
                            P1_chunk(wi + 2, qs1.pop(0))
                        if qs3:
                            P3_chunk(wi, qs3.pop(0))
            esb_prev = None
        else:
            start_P1(0)
            for q in range(nch[0]):
                P1_chunk(0, q)
            finish_P1(0)
            for wi in range(NW):
                nxt = wi + 1 < NW
                if nxt:
                    start_P1(wi + 1)
                qs1 = list(range(nch[wi + 1])) if nxt else []
                qs3 = list(range(nch[wi]))
                while qs1 or qs3:
                    if qs1:
                        P1_chunk(wi + 1, qs1.pop(0))
                    if qs3:
                        P3_chunk(wi, qs3.pop(0))
                if nxt:
                    finish_P1(wi + 1)

    nc.finalize()
    return nc


def _get_nc():
    if "nc" not in _cache:
        _cache["nc"] = _build_nc()
    return _cache["nc"]


def kernel(x, gamma, beta, _want_profile=False):
    import ml_dtypes
    from concourse.bass_utils import run_bass_kernel_spmd

    x = np.asarray(x, dtype=np.float32)
    gamma = np.asarray(gamma, dtype=np.float32)
    beta = np.asarray(beta, dtype=np.float32)
    assert x.shape == (B, T, C), x.shape

    cfg = DEFAULT_CFG
    lmatT, dmW, srowT, EW, rden, _ = _host_constants(
        max(cfg["win_sbs"]), cfg["win_sbs"]
    )
    bf = ml_dtypes.bfloat16
    x_bf = x.astype(bf)
    nc = _get_nc()

    in_maps = []
    for core in range(NCORES):
        xs = np.ascontiguousarray(x_bf[core * BPC : (core + 1) * BPC])
        in_maps.append(
            {
                "x": xs,
                "lmatT": lmatT.astype(bf),
                "dmW": dmW.astype(bf),
                "srowT": srowT.astype(bf),
                "EW": EW.astype(bf),
                "rden": rden,
            }
        )

    res = run_bass_kernel_spmd(nc, in_maps, list(range(NCORES)), trace=False)
    n = np.concatenate(
        [np.asarray(res.results[core]["y"]) for core in range(NCORES)], axis=0
    ).astype(np.float32)
    # host affine: y = x * (1 + (C*gamma)*n) + beta
    y = x * (1.0 + n * (np.float32(C) * gamma[None, :, :])) + beta[None, :, :]
    y = np.ascontiguousarray(y.astype(np.float32))
    if _want_profile:
        _cache["last_profile"] = res
    return y

